# revision 6
# baseline (speedup 1.0000x reference)
"""Trainium2 Bass kernel for nn_Dereverb_T60 (bidirectional GRU over sliding windows).

Problem structure (hardcoded from the reference):
  B=8, T=16000, STRIDE=16, H=16, t60=1000 samples -> C=1000 windows per sample,
  each window = 1000 steps of a 1-input GRU (fwd: 984 warmup + 16 collected steps;
  bwd: 16 steps from the end). Output = mean over hidden dim of (ys_f + ys_b).

Sharding: pure data parallel - core c processes sample b=c (1000 windows, padded
to 1024 SBUF columns). GRU weights replicated.

Hardware constraints honored: every compute-op AP starts at a 32-aligned
partition, and both tensor_tensor inputs share the same start partition. All
16-row GRU quantities therefore ride at +16 inside 32-row blocks with a junk
lane at +0 (zeros flow through the junk lanes), and the z gate is computed
twice (duplicated pre-activation columns) so r and z are each available at the
in-block offset their consumer needs.

Per-step pipeline (window slots on the free dim, n=1024):
  matmul (per-step weight-variant lhsT [97,128]) -> pg psum [128, n] with column
  blocks [pad|nh | pad|ni | zpre|rpre | pad|zpre2]; sigmoid -> [z|r|junk|z2];
  then tanh + 5 DVE tensor_tensor ops produce h' in rhs rows 80:96.
rhs rows: 0:16 x-slot-A, 32:48 x-slot-B (16-step input blocks, ping-pong DMA),
  64:80 scratch (zero-weighted junk lane), 80:96 h, 96 bias const 1.0.
Tail windows (j>=938): h column reset at step pad_j = 16j-15000 instead of
  masking (left-padding equivalence).
"""

import numpy as np
from contextlib import ExitStack

import concourse.bass as bass
import concourse.bacc as bacc
import concourse.mybir as mybir
import concourse.tile as tile
from concourse.bass_utils import run_bass_kernel_spmd

F32 = mybir.dt.float32
AF = mybir.ActivationFunctionType
OP = mybir.AluOpType

B, T, STRIDE, H, T60 = 8, 16000, 16, 16, 1000
C = T // STRIDE          # 1000 windows per sample
NSLOT = 1024             # padded window slots per core
NCORES = 8
S1 = T60 - STRIDE        # 984 warmup steps
KDIM = 97                # rhs rows (see module docstring)
HROW = 80                # h rows 80:96
SCR = 64                 # scratch block start (rows 64:96 = [scratch; h])
BROW = 96                # bias const-1.0 row
MDIM = 128               # gate columns (with pad/duplicate lanes)


def _emit_all(nc):
    xf = nc.dram_tensor("xf", [T60, NSLOT], F32, kind="ExternalInput").ap()
    xb = nc.dram_tensor("xb", [STRIDE, NSLOT], F32, kind="ExternalInput").ap()
    wvf = nc.dram_tensor("wvf", [KDIM, 32 * MDIM], F32, kind="ExternalInput").ap()
    wvb = nc.dram_tensor("wvb", [KDIM, 16 * MDIM], F32, kind="ExternalInput").ap()
    onesm = nc.dram_tensor("onesm", [32, 256], F32, kind="ExternalInput").ap()
    out = nc.dram_tensor("out", [16, NSLOT], F32, kind="ExternalOutput").ap()

    with tile.TileContext(nc) as tc, ExitStack() as ctx:
        const_pool = ctx.enter_context(tc.tile_pool(name="const", bufs=1))
        state_pool = ctx.enter_context(tc.tile_pool(name="state", bufs=1))
        pg_pool = ctx.enter_context(tc.tile_pool(name="pg", bufs=2, space="PSUM"))
        po_pool = ctx.enter_context(tc.tile_pool(name="po", bufs=1, space="PSUM"))

        wvf_sb = const_pool.tile([KDIM, 32 * MDIM], F32, tag="wvf")
        wvb_sb = const_pool.tile([KDIM, 16 * MDIM], F32, tag="wvb")
        ones_sb = const_pool.tile([96, 256], F32, tag="ones")
        rhs = state_pool.tile([KDIM, NSLOT], F32, tag="rhs")
        rz = state_pool.tile([64, NSLOT], F32, tag="rz")    # [z; r; junk; z2]
        sc = state_pool.tile([64, NSLOT], F32, tag="sc")    # rows 32:64 used
        ti = state_pool.tile([32, NSLOT], F32, tag="ti")
        tb = state_pool.tile([96, NSLOT], F32, tag="tb")    # rows 64:96 used
        yt = state_pool.tile([96, NSLOT], F32, tag="yt")    # rows 64:96 used
        osb = state_pool.tile([16, NSLOT], F32, tag="osb")
        po = po_pool.tile([16, NSLOT], F32, tag="po")

        nc.sync.dma_start(wvf_sb[:, :], wvf[:, :])
        nc.sync.dma_start(wvb_sb[:, :], wvb[:, :])
        nc.vector.memset(ones_sb[64:96, :], 0.0)
        nc.sync.dma_start(ones_sb[64:96, :], onesm[:, :])
        nc.vector.memset(rhs[0:64, :], 0.0)   # x slots + unused rows
        nc.vector.memset(rhs[64:96, :], 0.0)  # scratch + h
        nc.vector.memset(rhs[BROW:BROW + 1, :], 1.0)
        nc.sync.dma_start(rhs[0:16, :], xf[0:16, :])  # x block 0

        h32 = rhs[SCR:SCR + 32, :]  # [scratch; h]

        def step(wv_sb, nvar, k):
            vv = k % nvar
            pg = pg_pool.tile([MDIM, NSLOT], F32, tag="pg")
            lhs = wv_sb[:, MDIM * vv:MDIM * vv + MDIM]
            nc.tensor.matmul(pg[:, 0:512], lhs, rhs[:, 0:512])
            nc.tensor.matmul(pg[:, 512:1024], lhs, rhs[:, 512:1024])
            # rz = [z; r; junk; z2]
            nc.scalar.activation(rz[0:64, :], pg[64:128, :], AF.Sigmoid)
            # u = r*nh (rides at +16; junk lane +0 stays 0)
            nc.vector.tensor_tensor(sc[32:64, :], rz[0:32, :], pg[0:32, :], OP.mult)
            # ti = u + ni
            nc.vector.tensor_tensor(ti[0:32, :], sc[32:64, :], pg[32:64, :], OP.add)
            # t = tanh(ti)
            nc.scalar.activation(tb[64:96, :], ti[0:32, :], AF.Tanh)
            # w = h - t
            nc.vector.tensor_tensor(sc[32:64, :], h32, tb[64:96, :], OP.subtract)
            # y = z2 * w
            nc.vector.tensor_tensor(yt[64:96, :], rz[32:64, :], sc[32:64, :], OP.mult)
            # h' = y + t  (scratch lane: 0.5*scratch -> stays 0)
            nc.vector.tensor_tensor(h32, yt[64:96, :], tb[64:96, :], OP.add)

        def collect(i, start, stop):
            for hf in (0, 1):
                sl = slice(512 * hf, 512 * hf + 512)
                nc.tensor.matmul(po[:, sl],
                                 ones_sb[64:96, 16 * i:16 * i + 16],
                                 h32[:, sl], start=start, stop=stop)

        # ---------------- forward: 1000 steps ----------------
        for k in range(T60):
            if k % 16 == 0:
                lo = k + 16
                if lo < T60:
                    hi = min(lo + 16, T60)
                    srow = 32 * ((lo // 16) % 2)
                    nc.sync.dma_start(rhs[srow:srow + (hi - lo), :], xf[lo:hi, :])
            if k % 16 == 8 and k <= S1:
                j = (15000 + k) // 16
                nc.vector.memset(rhs[SCR:SCR + 32, j:j + 1], 0.0)
            step(wvf_sb, 32, k)
            if k >= S1:
                collect(k - S1, start=(k == S1), stop=False)

        # ---------------- backward: 16 steps ----------------
        nc.vector.memset(rhs[64:96, :], 0.0)
        nc.sync.dma_start(rhs[0:16, :], xb[:, :])
        nc.vector.memset(rhs[32:48, :], 0.0)  # clear stale fwd x-slot-B
        for k in range(STRIDE):
            step(wvb_sb, 16, k)
            collect(STRIDE - 1 - k, start=False, stop=(k == STRIDE - 1))

        # psum -> sbuf -> dram
        nc.vector.tensor_copy(osb[:, :], po[:, :])
        nc.sync.dma_start(out[:, :], osb[:, :])


def build():
    nc = bacc.Bacc("TRN2", target_bir_lowering=False, debug=False,
                   num_devices=NCORES)
    _emit_all(nc)
    nc.compile()
    return nc


# ---------------------------------------------------------------------------
# host-side packing
# ---------------------------------------------------------------------------
# pg column blocks:   0:16 PAD | 16:32 nh | 32:48 PAD | 48:64 ni
#                    64:80 zpre | 80:96 rpre | 96:112 PAD | 112:128 zpre2
# rhs rows: 0:16 xA | 16:32 0 | 32:48 xB | 48:64 0 | 64:80 scratch | 80:96 h
#           | 96 bias

def _pack_weight_variants(w_ih, w_hh, b_ih, b_hh, nvar):
    w_ih = np.asarray(w_ih, np.float32).reshape(3 * H)
    w_hh = np.asarray(w_hh, np.float32)
    b_ih = np.asarray(b_ih, np.float32)
    b_hh = np.asarray(b_hh, np.float32)
    wv = np.zeros((KDIM, nvar * MDIM), np.float32)
    for vv in range(nvar):
        blk = np.zeros((KDIM, MDIM), np.float32)
        xr = vv if vv < 16 else 32 + (vv - 16)
        blk[xr, 48:64] = w_ih[32:48]     # ni
        blk[xr, 64:80] = w_ih[16:32]     # zpre
        blk[xr, 80:96] = w_ih[0:16]      # rpre
        blk[xr, 112:128] = w_ih[16:32]   # zpre2
        blk[BROW, 16:32] = b_hh[32:48]                  # nh
        blk[BROW, 48:64] = b_ih[32:48]                  # ni
        blk[BROW, 64:80] = b_ih[16:32] + b_hh[16:32]    # zpre
        blk[BROW, 80:96] = b_ih[0:16] + b_hh[0:16]      # rpre
        blk[BROW, 112:128] = b_ih[16:32] + b_hh[16:32]  # zpre2
        blk[HROW:HROW + 16, 16:32] = w_hh[32:48, :].T    # nh
        blk[HROW:HROW + 16, 64:80] = w_hh[16:32, :].T    # zpre
        blk[HROW:HROW + 16, 80:96] = w_hh[0:16, :].T     # rpre
        blk[HROW:HROW + 16, 112:128] = w_hh[16:32, :].T  # zpre2
        wv[:, MDIM * vv:MDIM * vv + MDIM] = blk
    return wv


def _pack_inputs(inputs):
    inp = np.asarray(inputs["input"], np.float32)
    wvf = _pack_weight_variants(inputs["w_ih_f"], inputs["w_hh_f"],
                                inputs["b_ih_f"], inputs["b_hh_f"], 32)
    wvb = _pack_weight_variants(inputs["w_ih_b"], inputs["w_hh_b"],
                                inputs["b_ih_b"], inputs["b_hh_b"], 16)
    # collect matrix: lhsT [32,16] per i; rows 16:32 (the h rows of the
    # rhs[64:96] block) hold (1/16)*e_i -> psum row i = mean_h(h)
    onesm = np.zeros((32, 256), np.float32)
    for i in range(16):
        onesm[16:32, 16 * i + i] = 1.0 / 16.0

    j = np.arange(938)
    k = np.arange(T60)
    kb = np.arange(STRIDE)
    in_maps = []
    for c in range(NCORES):
        flp = inp[c, ::-1]
        XF = np.zeros((T60, NSLOT), np.float32)
        XF[:, :938] = flp[16 * j[None, :] + k[:, None]]
        XF[:, 938:1000] = flp[15000 + k][:, None]
        XB = np.zeros((STRIDE, NSLOT), np.float32)
        XB[:, :938] = flp[16 * j[None, :] + 999 - kb[:, None]]
        XB[:, 938:1000] = flp[15999 - kb][:, None]
        in_maps.append({"xf": XF, "xb": XB, "wvf": wvf, "wvb": wvb,
                        "onesm": onesm})
    return in_maps


_NC_CACHE = []


def kernel(**inputs):
    if not _NC_CACHE:
        _NC_CACHE.append(build())
    nc = _NC_CACHE[0]
    in_maps = _pack_inputs(inputs)
    res = run_bass_kernel_spmd(nc, in_maps, list(range(NCORES)))
    out = np.zeros((B, T), np.float32)
    for c in range(NCORES):
        arr = res.results[c]["out"][:, :C]        # [16, 1000]
        out[c] = arr.T.reshape(T)[::-1]
    return out
